# revision 47
# baseline (speedup 1.0000x reference)
"""GCN (2-layer GCNConv + global_add_pool + dense head) on 8 TRN2 cores.

Strategy (graph/data parallel, per sharding hint):
 - Nodes block-partitioned: core c owns rows [6250c, 6250(c+1)).
 - Edges partitioned by destination block, sorted by (dst window, src half).
 - Weight-commute: segment_sum((x*dinv)[src]) @ W == segment_sum(h*dinv), so
   each layer gathers PRE-matmul scaled features and applies W after
   aggregation.  The layer-1 gather table is therefore a host input (xtab,
   permuted [(core,p,w) subrow] layout) -- no layer-1 AllGather, gathers
   start immediately.  Layer 2 AllGathers x2s = relu(.)*dinv built in the
   layer-1 epilogue.
 - Aggregation: batched dma_gather (one 512B descriptor per edge) using an
   overlapping-stride table view (elem_step = 1 row, elem = 2 rows) so each
   slot holds its true source row; int16 index range handled by splitting
   the table into lo/hi half views; per-batch num_idxs trimmed to the
   max-over-cores edge count with a matching K-cut on the final one-hot
   matmul.  Per 128-slot chunk one matmul accumulates [feat x dst] in PSUM
   (lhsT = gathered rows, rhs = one-hot C built on DVE via is_equal);
   self-loop via identity matmul of the transposed feature buffer.
 - Epilogue per window: copy PSUM->SBUF fp16, matmul by W (+ sqrt(deg) x b
   bias matmul), relu with dinv (layer 1: dinv^2, since relu(z)*d =
   relu(z*d)) on Act; layer 2 accumulates global_add_pool via one-hot graph
   matmuls.
 - Pooled partials are scattered to graph rows and multiplied by Wh (+bh/8)
   BEFORE the fp16 AllReduce of logits; log_softmax (no max shift; logits
   are O(5)) runs redundantly on every core.
"""
import sys

sys.path.insert(0, "/opt/trn_rl_repo")

import math
import numpy as np

import concourse.bacc as bacc
import concourse.bass as bass
import concourse.mybir as mybir
import concourse.tile as tile

P = 128
N_NODES = 50000
N_EDGES = 640000
DIM = 128
DIM_OUT = 64
NUM_GRAPHS = 512
NCORES = 8
NB = N_NODES // NCORES          # 6250 nodes per core
WPC = math.ceil(NB / P)         # 49 windows per core
NBP = WPC * P                   # 6272 padded
HALFP = 25088                   # permuted-subrow split (= 512 * WPC)
TPR = NCORES * P                # 1024 table partition-rows
WGRP = 2                        # windows per gather batch

fp32 = mybir.dt.float32
fp16 = mybir.dt.float16
i16 = mybir.dt.int16


def make_batches():
    sizes = [1]
    while sum(sizes) + WGRP <= WPC - 4:
        sizes.append(WGRP)
    while sum(sizes) < WPC:
        sizes.append(1)
    out = []
    w0 = 0
    for nw in sizes:
        out.append((w0, nw))
        w0 += nw
    return out


# ---------------------------------------------------------------- host prep
def preprocess(x, edge_index, x_batch):
    src = np.asarray(edge_index[0], dtype=np.int64)
    dst = np.asarray(edge_index[1], dtype=np.int64)
    xb = np.asarray(x_batch, dtype=np.int64)
    x = np.asarray(x, dtype=np.float32)

    edeg = np.bincount(dst, minlength=N_NODES)
    deg = 1.0 + edeg.astype(np.float32)
    dinv = (1.0 / np.sqrt(deg)).astype(np.float32)
    sqd = np.sqrt(deg).astype(np.float32)

    order = np.argsort(dst, kind="stable")
    src_sorted = src[order]
    starts = np.zeros(N_NODES + 1, np.int64)
    np.cumsum(edeg, out=starts[1:])

    # per (core, window, half) edge lists; chunk grid = max over cores
    lists = [[None] * WPC for _ in range(NCORES)]
    cwlo = np.zeros((NCORES, WPC), np.int64)
    cwhi = np.zeros((NCORES, WPC), np.int64)
    for c in range(NCORES):
        b = c * NB
        for w in range(WPC):
            lo, hi = b + w * P, b + min((w + 1) * P, NB)
            srcs_w = src_sorted[starts[lo]:starts[hi]]
            nloc_w = np.repeat(np.arange(hi - lo), edeg[lo:hi])
            # permuted table subrow: node n -> (core, part, window) subrow id
            cc, rr = srcs_w // NB, srcs_w % NB
            pr = (cc * P + rr % P) * WPC + rr // P
            m = pr < HALFP
            lists[c][w] = (pr[m], nloc_w[m], pr[~m] - HALFP, nloc_w[~m])
            cwlo[c, w] = max(1, math.ceil(int(m.sum()) / P))
            cwhi[c, w] = max(1, math.ceil(int((~m).sum()) / P))
    E = {"lo": np.zeros((NCORES, WPC), np.int64),
         "hi": np.zeros((NCORES, WPC), np.int64)}
    for c in range(NCORES):
        for w in range(WPC):
            E["lo"][c, w] = len(lists[c][w][0])
            E["hi"][c, w] = len(lists[c][w][2])

    # packed per-batch streams: windows concatenated with no alignment;
    # each window processes chunk range [min-core start, max-core end)
    batches = make_batches()
    meta = {}
    for half in ("lo", "hi"):
        mb_list = []
        c0 = nob = 0
        for (w0, nw) in batches:
            cums = np.zeros((NCORES, nw + 1), np.int64)
            cums[:, 1:] = np.cumsum(E[half][:, w0:w0 + nw], axis=1)
            tmax = int(cums[:, nw].max())
            nidx = max(16, -(-tmax // 16) * 16)
            ncols = -(-nidx // 128)
            wins = []
            for i in range(nw):
                s_i = min(int(cums[:, i].min()) // 128, ncols - 1)
                e_i = max(min(-(-int(cums[:, i + 1].max()) // 128), ncols),
                          s_i + 1)
                wins.append((s_i, e_i, nob))
                nob += e_i - s_i
            mb_list.append(dict(c0=c0, ncols=ncols, nidx=nidx, wins=wins,
                                klast=nidx - 128 * (ncols - 1)))
            c0 += ncols
        meta[half] = dict(batches=mb_list, TC=c0, NOC=nob)

    def wrap16(flat):
        # index i -> [i % 16, i // 16], replicated across 128 partitions
        n = len(flat)
        arr = np.zeros((P, n // 16), np.int16)
        arr[:16] = flat.reshape(n // 16, 16).T
        for r in range(1, 8):
            arr[16 * r:16 * (r + 1)] = arr[:16]
        return arr

    per_core = []
    for c in range(NCORES):
        b = c * NB
        streams = {}
        for half, ilo in (("lo", 0), ("hi", 2)):
            m = meta[half]
            idxf = np.zeros(m["TC"] * P, np.int16)
            nof = np.full(m["NOC"] * P, -1.0, np.float32)
            for (w0, nw), mb in zip(batches, m["batches"]):
                base = mb["c0"] * P
                pos = 0
                for i in range(nw):
                    s, nl = lists[c][w0 + i][ilo], lists[c][w0 + i][ilo + 1]
                    ln = len(s)
                    idxf[base + pos:base + pos + ln] = s.astype(np.int16)
                    s_i, e_i, nob_i = mb["wins"][i]
                    pa = np.arange(pos, pos + ln)
                    tgt = (nob_i + pa // P - s_i) * P + pa % P
                    nof[tgt] = nl
                    pos += ln
            streams[f"idx_{half}"] = wrap16(idxf)
            streams[f"no_{half}"] = nof.reshape(m["NOC"], P).T.copy()

        nid = b + np.arange(NBP)
        ok = np.arange(NBP) < NB
        nidc = np.minimum(nid, N_NODES - 1)
        dinv_c = np.where(ok, dinv[nidc], 0.0).astype(np.float32)
        sqd_c = np.where(ok, sqd[nidc], 0.0).astype(np.float16)
        gmin = int(xb[b])
        xbs_c = np.where(ok, xb[nidc] - gmin, 200.0).astype(np.float32)
        assert int(xb[b + NB - 1]) - gmin + 1 <= P
        pools = np.stack(
            [gmin + np.arange(P, dtype=np.float32) - P * b4 for b4 in range(4)],
            axis=1,
        ).astype(np.float32)

        xs_c = x[b:b + NB] * dinv[b:b + NB, None]
        xT = np.zeros((DIM, NBP), np.float16)
        xT[:, :NB] = xs_c.T.astype(np.float16)

        per_core.append(dict(
            xT=xT,
            dinv2d=dinv_c.reshape(WPC, P).T.copy(),
            dinvsq=(dinv_c * dinv_c).reshape(WPC, P).T.copy(),
            sqdT=sqd_c.reshape(1, NBP),
            xbshift=xbs_c.reshape(WPC, P).T.copy(),
            pools=pools,
            **streams,
        ))

    # full scaled-feature table in permuted layout (same for every core)
    xall = (x * dinv[:, None]).astype(np.float16)
    xtab = np.zeros((TPR + 1, NBP), np.float16)
    n = np.arange(N_NODES)
    cc, rr = n // NB, n % NB
    rows = cc * P + rr % P
    cols = (rr // P) * P
    xtab[rows[:, None], cols[:, None] + np.arange(P)[None, :]] = xall
    for pc in per_core:
        pc["xtab"] = xtab

    shared = dict(meta=meta)
    return per_core, shared


def const_inputs(W1, b1, W2, b2, Wh, bh):
    iota = np.tile(np.arange(P, dtype=np.float32)[None, :], (P, 1))
    return dict(
        iota=iota, iota16=iota.astype(np.float16),
        ident16=np.eye(P, dtype=np.float16),
        ident=np.eye(P, dtype=np.float32),
        W1=np.asarray(W1, np.float16), W2=np.asarray(W2, np.float16),
        Wh=np.asarray(Wh, np.float32),
        b1=np.asarray(b1, np.float16).reshape(1, DIM),
        b2=np.asarray(b2, np.float16).reshape(1, DIM),
        bh8=np.asarray(bh, np.float32).reshape(1, DIM_OUT) / NCORES,
        ones512=np.ones((1, NUM_GRAPHS), np.float32),
    )


# ---------------------------------------------------------------- kernel
def build_kernel(shared, single_core=False):
    meta = shared["meta"]
    TClo, TChi = meta["lo"]["TC"], meta["hi"]["TC"]
    NOClo, NOChi = meta["lo"]["NOC"], meta["hi"]["NOC"]

    nc = bacc.Bacc("TRN2", target_bir_lowering=False, debug=False,
                   enable_asserts=False,
                   num_devices=1 if single_core else NCORES)

    # inputs
    d_xT = nc.dram_tensor("xT", [DIM, NBP], fp16, kind="ExternalInput")
    d_idx = {h: nc.dram_tensor(f"idx_{h}", [P, tc * 8], i16,
                               kind="ExternalInput")
             for h, tc in (("lo", TClo), ("hi", TChi))}
    d_no = {h: nc.dram_tensor(f"no_{h}", [P, tc], fp32, kind="ExternalInput")
            for h, tc in (("lo", NOClo), ("hi", NOChi))}
    d_dinv = nc.dram_tensor("dinv2d", [P, WPC], fp32, kind="ExternalInput")
    d_dinvsq = nc.dram_tensor("dinvsq", [P, WPC], fp32, kind="ExternalInput")
    d_xtab = nc.dram_tensor("xtab", [TPR + 1, NBP], fp16,
                            kind="ExternalInput")
    d_sqd = nc.dram_tensor("sqdT", [1, NBP], fp16, kind="ExternalInput")
    d_xbs = nc.dram_tensor("xbshift", [P, WPC], fp32, kind="ExternalInput")
    d_pools = nc.dram_tensor("pools", [P, 4], fp32, kind="ExternalInput")
    d_iota = nc.dram_tensor("iota", [P, P], fp32, kind="ExternalInput")
    d_iota16 = nc.dram_tensor("iota16", [P, P], fp16, kind="ExternalInput")
    d_id16 = nc.dram_tensor("ident16", [P, P], fp16, kind="ExternalInput")
    d_W = [nc.dram_tensor("W1", [DIM, DIM], fp16, kind="ExternalInput"),
           nc.dram_tensor("W2", [DIM, DIM], fp16, kind="ExternalInput")]
    d_b = [nc.dram_tensor("b1", [1, DIM], fp16, kind="ExternalInput"),
           nc.dram_tensor("b2", [1, DIM], fp16, kind="ExternalInput")]
    d_Wh = nc.dram_tensor("Wh", [DIM, DIM_OUT], fp32, kind="ExternalInput")
    d_bh = nc.dram_tensor("bh8", [1, DIM_OUT], fp32, kind="ExternalInput")
    d_ones = nc.dram_tensor("ones512", [1, NUM_GRAPHS], fp32,
                            kind="ExternalInput")

    d_out = nc.dram_tensor("out", [NUM_GRAPHS, DIM_OUT], fp32,
                           kind="ExternalOutput")

    # internal DRAM (layer-2 table in permuted [core*P+p, w*DIM+f] layout;
    # the layer-1 table is the host-provided xtab input)
    tbl = nc.dram_tensor("table1", [TPR + 1, NBP], fp16, addr_space="Shared")
    ag_in = nc.dram_tensor("ag_in1", [P, NBP], fp16)
    ar_in = nc.dram_tensor("ar_in", [NUM_GRAPHS, DIM_OUT], fp16)
    ar_out = nc.dram_tensor("ar_out", [NUM_GRAPHS, DIM_OUT], fp16,
                            addr_space="Shared")

    # gather batches: [(w0, nw, col0_lo, cols_lo, col0_hi, cols_hi)]
    batches = make_batches()

    with tile.TileContext(nc) as tc:
        with tc.tile_pool(name="const", bufs=1) as cst, \
             tc.tile_pool(name="big", bufs=1) as bigp, \
             tc.tile_pool(name="glo", bufs=8) as glo_pool, \
             tc.tile_pool(name="ghi", bufs=8) as ghi_pool, \
             tc.tile_pool(name="cpool", bufs=12) as cpool, \
             tc.tile_pool(name="work", bufs=6) as wk, \
             tc.tile_pool(name="ps_feat", bufs=3, space="PSUM") as ps_feat, \
             tc.tile_pool(name="ps_out", bufs=4, space="PSUM") as ps_out, \
             tc.tile_pool(name="ps_aux", bufs=1, space="PSUM") as ps_aux:

            # ---- constants / inputs to SBUF (feature-phase deps first)
            xT_sb = bigp.tile([DIM, NBP], fp16)
            nc.sync.dma_start(xT_sb[:, 0:7 * P], d_xT[:, 0:7 * P])
            nc.sync.dma_start(xT_sb[:, 7 * P:], d_xT[:, 7 * P:])
            W_sb = []
            for l in range(2):
                t = cst.tile([DIM, DIM], fp16, name=f"W{l}_sb")
                nc.sync.dma_start(t[:], d_W[l][:, :])
                W_sb.append(t)
            dinv_sb = cst.tile([P, WPC], fp32)
            nc.sync.dma_start(dinv_sb[:], d_dinv[:, :])
            dinvsq_sb = cst.tile([P, WPC], fp32)
            nc.sync.dma_start(dinvsq_sb[:], d_dinvsq[:, :])
            idx_sb = {}
            no_sb = {}
            for h, tc_, noc_ in (("lo", TClo, NOClo), ("hi", TChi, NOChi)):
                t = bigp.tile([P, tc_ * 8], i16, name=f"idx{h}_sb")
                nc.sync.dma_start(t[:], d_idx[h][:, :])
                idx_sb[h] = t
                t = bigp.tile([P, noc_], fp32, name=f"no{h}_sb")
                nc.sync.dma_start(t[:], d_no[h][:, :])
                no_sb[h] = t
            sqd_sb = cst.tile([1, NBP], fp16)
            nc.sync.dma_start(sqd_sb[:], d_sqd[:, :])
            xbs_sb = cst.tile([P, WPC], fp32)
            nc.sync.dma_start(xbs_sb[:], d_xbs[:, :])
            pools_sb = cst.tile([P, 4], fp32)
            nc.sync.dma_start(pools_sb[:], d_pools[:, :])
            iota_sb = cst.tile([P, P], fp32)
            nc.sync.dma_start(iota_sb[:], d_iota[:, :])
            iota16_sb = cst.tile([P, P], fp16)
            nc.sync.dma_start(iota16_sb[:], d_iota16[:, :])
            id16_sb = cst.tile([P, P], fp16)
            nc.sync.dma_start(id16_sb[:], d_id16[:, :])
            b_sb = []
            for l in range(2):
                t = cst.tile([1, DIM], fp16, name=f"b{l}_sb")
                nc.sync.dma_start(t[:], d_b[l][:, :])
                b_sb.append(t)
            Wh_sb = cst.tile([DIM, DIM_OUT], fp32)
            nc.sync.dma_start(Wh_sb[:], d_Wh[:, :])
            bh_sb = cst.tile([1, DIM_OUT], fp32)
            nc.sync.dma_start(bh_sb[:], d_bh[:, :])
            ones_sb = cst.tile([1, NUM_GRAPHS], fp32)
            nc.sync.dma_start(ones_sb[:], d_ones[:, :])

            x2s_sb = bigp.tile([P, NBP], fp16, name="x2s")
            x2sT_sb = bigp.tile([P, NBP], fp16, name="x2sT")

            # dummy Ln+Exp up front: forces the all-in-one act-func table
            # (natural_log_exp_and_others) to load once, off the critical path
            dum = cst.tile([1, 1], fp32)
            nc.vector.memset(dum[:], 1.0)
            nc.scalar.activation(out=dum[:], in_=dum[:],
                                 func=mybir.ActivationFunctionType.Ln)
            nc.scalar.activation(out=dum[:], in_=dum[:],
                                 func=mybir.ActivationFunctionType.Exp)

            # subrow gather views: one 256B element per edge
            gview = {}
            for h, base in (("lo", 0), ("hi", NCORES * P // 2)):
                gview[h] = [
                    bass.AP(t[base:, :].tensor, t[base:, :].offset,
                            [[DIM, HALFP + 1], [1, DIM]])
                    for t in (d_xtab, tbl)
                ]

            AGB = [6, 13, 20, 27, 34, 41, 45, 48]

            def ag_flush(w):
                # flush x2s windows to ag_in in groups (big descriptors)
                if w in AGB:
                    w0_ = AGB[AGB.index(w) - 1] + 1 if w != 6 else 0
                    gsl = slice(w0_ * P, (w + 1) * P)
                    nc.sync.dma_start(ag_in[:, gsl], x2s_sb[:, gsl])
                if w == WPC - 1:
                    if single_core:
                        nc.sync.dma_start(tbl[0:P, :], ag_in[:, :])
                    else:
                        nc.gpsimd.collective_compute(
                            "AllGather", mybir.AluOpType.bypass,
                            ins=[ag_in[:, :]],
                            outs=[tbl[0:TPR, :]],
                            replica_groups=[list(range(NCORES))])

            # pooling scatter one-hots depend only on constants: prebuild
            S4 = cst.tile([P, 4, P], fp32)
            for b4 in range(4):
                nc.vector.tensor_scalar(
                    out=S4[:, b4, :], in0=iota_sb[:],
                    scalar1=pools_sb[:, b4:b4 + 1],
                    scalar2=None, op0=mybir.AluOpType.is_equal)

            # ---- aggregation layers
            pool_ps = ps_aux.tile([P, DIM], fp32, space="PSUM", tag="aux")

            for l in range(2):
                for bi, (w0, nw) in enumerate(batches):
                    gt = {}
                    for h, pool_ in (("lo", glo_pool), ("hi", ghi_pool)):
                        mb = meta[h]["batches"][bi]
                        g = pool_.tile([P, mb["ncols"], DIM], fp16,
                                       tag="g", name=f"g{h}_{l}_{w0}")
                        c0 = mb["c0"]
                        nidx = mb["nidx"]
                        nc.gpsimd.dma_gather(
                            out_ap=g[:, :, :], in_ap=gview[h][l],
                            idxs_ap=idx_sb[h][:, 8 * c0:8 * c0 + nidx // 16],
                            num_idxs=nidx, num_idxs_reg=nidx,
                            elem_size=DIM, elem_step=DIM,
                            single_packet=False)
                        gt[h] = g

                    for w in range(w0, w0 + nw):
                        sl = slice(w * P, (w + 1) * P)
                        # poT accumulates [feat x dst] (transposed)
                        po = ps_out.tile([P, DIM], fp32, space="PSUM", tag="po")
                        selfT = xT_sb if l == 0 else x2sT_sb
                        nc.tensor.matmul(out=po[:], lhsT=id16_sb[:],
                                         rhs=selfT[:, sl],
                                         start=True, stop=False)
                        for h in ("lo", "hi"):
                            mb = meta[h]["batches"][bi]
                            s_i, e_i, nob_i = mb["wins"][w - w0]
                            last = h == "hi"
                            for j in range(s_i, e_i):
                                k = (mb["klast"] if j == mb["ncols"] - 1
                                     else P)
                                nocol = nob_i + j - s_i
                                C = cpool.tile([P, P], fp16, tag="C")
                                nc.vector.tensor_scalar(
                                    out=C[0:k, :], in0=iota16_sb[0:k, :],
                                    scalar1=no_sb[h][0:k, nocol:nocol + 1],
                                    scalar2=None,
                                    op0=mybir.AluOpType.is_equal)
                                nc.tensor.matmul(
                                    out=po[:], lhsT=gt[h][0:k, j, 0:DIM],
                                    rhs=C[0:k, :],
                                    start=False,
                                    stop=last and (j == e_i - 1))
                        # epilogue: apply W post-aggregation
                        posb = wk.tile([P, DIM], fp16, tag="posb")
                        nc.scalar.activation(
                            out=posb[:], in_=po[:],
                            func=mybir.ActivationFunctionType.Copy)
                        ph2 = ps_feat.tile([P, DIM], fp32, space="PSUM",
                                           tag="ph")
                        nc.tensor.matmul(out=ph2[:], lhsT=posb[:],
                                         rhs=W_sb[l][:],
                                         start=True, stop=False)
                        nc.tensor.matmul(out=ph2[:],
                                         lhsT=sqd_sb[0:1, sl],
                                         rhs=b_sb[l][:], start=False,
                                         stop=True)
                        if l == 0:
                            # x2s = relu(ph*dinv + b)*dinv = relu(ph*dinv^2
                            #       + sqd*b*dinv^2)   (dinv > 0)
                            nc.scalar.activation(
                                out=x2s_sb[:, sl], in_=ph2[:],
                                func=mybir.ActivationFunctionType.Relu,
                                scale=dinvsq_sb[:, w:w + 1])
                            ptr = ps_feat.tile([P, DIM], fp16, space="PSUM",
                                               tag="ph")
                            nc.tensor.transpose(out=ptr[:],
                                                in_=x2s_sb[:, sl],
                                                identity=id16_sb[:])
                            nc.scalar.activation(
                                out=x2sT_sb[:, sl], in_=ptr[:],
                                func=mybir.ActivationFunctionType.Copy)
                            ag_flush(w)
                        else:
                            xn = wk.tile([P, DIM], fp16, tag="xn")
                            nc.scalar.activation(
                                out=xn[:], in_=ph2[:],
                                func=mybir.ActivationFunctionType.Relu,
                                scale=dinv_sb[:, w:w + 1])
                            Cg = wk.tile([P, P], fp16, tag="Cg")
                            nc.vector.tensor_scalar(
                                out=Cg[:], in0=iota16_sb[:],
                                scalar1=xbs_sb[:, w:w + 1],
                                scalar2=None, op0=mybir.AluOpType.is_equal)
                            nc.tensor.matmul(out=pool_ps[:], lhsT=Cg[:],
                                             rhs=xn[:], start=(w == 0),
                                             stop=(w == WPC - 1))

            # ---- pooling scatter + AllReduce
            pool_sb = wk.tile([P, DIM], fp32)
            nc.vector.tensor_copy(pool_sb[:], pool_ps[:])
            lgi4 = wk.tile([P, 4, DIM_OUT], fp16)
            for b4 in range(4):
                # [feat x graph-block] scatter, then partial logits pre-AR
                pblk = ps_feat.tile([P, P], fp32, space="PSUM", tag="ph")
                nc.tensor.matmul(out=pblk[:], lhsT=pool_sb[:],
                                 rhs=S4[:, b4, :], start=True, stop=True)
                sblk = wk.tile([P, P], fp32, tag="sblk")
                nc.vector.tensor_copy(sblk[:], pblk[:])
                lp = ps_feat.tile([P, DIM_OUT], fp32, space="PSUM", tag="ph")
                nc.tensor.matmul(out=lp[:], lhsT=sblk[:],
                                 rhs=Wh_sb[:], start=True, stop=False)
                nc.tensor.matmul(out=lp[:], lhsT=ones_sb[0:1, 0:P],
                                 rhs=bh_sb[:], start=False, stop=True)
                nc.vector.tensor_copy(lgi4[:, b4, :], lp[:])
            nc.sync.dma_start(
                ar_in[:, :].rearrange("(b p) c -> p b c", p=P), lgi4[:])
            if single_core:
                nc.sync.dma_start(ar_out[:, :], ar_in[:, :])
            else:
                nc.gpsimd.collective_compute(
                    "AllReduce", mybir.AluOpType.add,
                    ins=[ar_in[:, :]], outs=[ar_out[:, :]],
                    replica_groups=[list(range(NCORES))])

            # ---- head: log_softmax on reduced logits
            lg4 = wk.tile([P, 4, DIM_OUT], fp16)
            nc.sync.dma_start(
                lg4[:], ar_out[:, :].rearrange("(b p) c -> p b c", p=P))
            e4 = wk.tile([P, 4, DIM_OUT], fp32)
            se4 = wk.tile([P, 4], fp32)
            lse4 = wk.tile([P, 4], fp32)
            o4 = wk.tile([P, 4, DIM_OUT], fp32)
            # logits are O(5) here, so exp without max-shift is safe
            nc.scalar.activation(out=e4[:].rearrange("p a b -> p (a b)"),
                                 in_=lg4[:].rearrange("p a b -> p (a b)"),
                                 func=mybir.ActivationFunctionType.Exp)
            nc.vector.tensor_reduce(out=se4[:], in_=e4[:],
                                    op=mybir.AluOpType.add,
                                    axis=mybir.AxisListType.X)
            nc.scalar.activation(out=lse4[:], in_=se4[:],
                                 func=mybir.ActivationFunctionType.Ln)
            for b4 in range(4):
                nc.vector.tensor_scalar(out=o4[:, b4, :], in0=lg4[:, b4, :],
                                        scalar1=lse4[:, b4:b4 + 1],
                                        scalar2=None,
                                        op0=mybir.AluOpType.subtract)
            nc.sync.dma_start(d_out[:, :].rearrange("(b p) f -> p b f", p=P),
                              o4[:])

    nc.compile()
    return nc


# ---------------------------------------------------------------- entry
def kernel(x, edge_index, x_batch, W1, b1, W2, b2, Wh, bh):
    """Full-input GCN kernel: shards nodes/edges across 8 NeuronCores."""
    from concourse.bass_utils import run_bass_kernel_spmd

    per_core, shared = preprocess(x, edge_index, x_batch)
    consts = const_inputs(W1, b1, W2, b2, Wh, bh)
    in_maps = [{**pc, **consts} for pc in per_core]
    nc = build_kernel(shared)
    declared = set()
    for alloc in nc.m.functions[0].allocations:
        if isinstance(alloc, mybir.MemoryLocationSet) and \
                alloc.kind == "ExternalInput":
            declared.add(alloc.memorylocations[0].name)
    in_maps = [{k: v for k, v in m.items() if k in declared} for m in in_maps]
    res = run_bass_kernel_spmd(nc, in_maps, core_ids=list(range(NCORES)))
    return np.asarray(res.results[0]["out"], dtype=np.float32)


# revision 48
# speedup vs baseline: 1.0174x; 1.0174x over previous
"""GCN (2-layer GCNConv + global_add_pool + dense head) on 8 TRN2 cores.

Strategy (graph/data parallel, per sharding hint):
 - Nodes block-partitioned: core c owns rows [6250c, 6250(c+1)).
 - Edges partitioned by destination block, sorted by (dst window, src half).
 - Weight-commute: segment_sum((x*dinv)[src]) @ W == segment_sum(h*dinv), so
   each layer gathers PRE-matmul scaled features and applies W after
   aggregation.  The layer-1 gather table is therefore a host input (xtab,
   permuted [(core,p,w) subrow] layout) -- no layer-1 AllGather, gathers
   start immediately.  Layer 2 AllGathers x2s = relu(.)*dinv built in the
   layer-1 epilogue.
 - Aggregation: batched dma_gather (one 512B descriptor per edge) using an
   overlapping-stride table view (elem_step = 1 row, elem = 2 rows) so each
   slot holds its true source row; int16 index range handled by splitting
   the table into lo/hi half views; per-batch num_idxs trimmed to the
   max-over-cores edge count with a matching K-cut on the final one-hot
   matmul.  Per 128-slot chunk one matmul accumulates [feat x dst] in PSUM
   (lhsT = gathered rows, rhs = one-hot C built on DVE via is_equal);
   self-loop via identity matmul of the transposed feature buffer.
 - Epilogue per window: copy PSUM->SBUF fp16, matmul by W (+ sqrt(deg) x b
   bias matmul), relu with dinv (layer 1: dinv^2, since relu(z)*d =
   relu(z*d)) on Act; layer 2 accumulates global_add_pool via one-hot graph
   matmuls.
 - Pooled partials are scattered to graph rows and multiplied by Wh (+bh/8)
   BEFORE the fp16 AllReduce of logits; log_softmax (no max shift; logits
   are O(5)) runs redundantly on every core.
"""
import sys

sys.path.insert(0, "/opt/trn_rl_repo")

import math
import numpy as np

import concourse.bacc as bacc
import concourse.bass as bass
import concourse.mybir as mybir
import concourse.tile as tile

P = 128
N_NODES = 50000
N_EDGES = 640000
DIM = 128
DIM_OUT = 64
NUM_GRAPHS = 512
NCORES = 8
NB = N_NODES // NCORES          # 6250 nodes per core
WPC = math.ceil(NB / P)         # 49 windows per core
NBP = WPC * P                   # 6272 padded
HALFP = 25088                   # permuted-subrow split (= 512 * WPC)
TPR = NCORES * P                # 1024 table partition-rows
WGRP = 2                        # windows per gather batch

fp32 = mybir.dt.float32
fp16 = mybir.dt.float16
i16 = mybir.dt.int16


def make_batches():
    sizes = [1]
    while sum(sizes) + WGRP <= WPC - 4:
        sizes.append(WGRP)
    while sum(sizes) < WPC:
        sizes.append(1)
    out = []
    w0 = 0
    for nw in sizes:
        out.append((w0, nw))
        w0 += nw
    return out


# ---------------------------------------------------------------- host prep
def preprocess(x, edge_index, x_batch):
    src = np.asarray(edge_index[0], dtype=np.int64)
    dst = np.asarray(edge_index[1], dtype=np.int64)
    xb = np.asarray(x_batch, dtype=np.int64)
    x = np.asarray(x, dtype=np.float32)

    edeg = np.bincount(dst, minlength=N_NODES)
    deg = 1.0 + edeg.astype(np.float32)
    dinv = (1.0 / np.sqrt(deg)).astype(np.float32)
    sqd = np.sqrt(deg).astype(np.float32)

    order = np.argsort(dst, kind="stable")
    src_sorted = src[order]
    starts = np.zeros(N_NODES + 1, np.int64)
    np.cumsum(edeg, out=starts[1:])

    # per (core, window, half) edge lists; chunk grid = max over cores
    lists = [[None] * WPC for _ in range(NCORES)]
    cwlo = np.zeros((NCORES, WPC), np.int64)
    cwhi = np.zeros((NCORES, WPC), np.int64)
    for c in range(NCORES):
        b = c * NB
        for w in range(WPC):
            lo, hi = b + w * P, b + min((w + 1) * P, NB)
            srcs_w = src_sorted[starts[lo]:starts[hi]]
            nloc_w = np.repeat(np.arange(hi - lo), edeg[lo:hi])
            # permuted table subrow: node n -> (core, part, window) subrow id
            cc, rr = srcs_w // NB, srcs_w % NB
            pr = (cc * P + rr % P) * WPC + rr // P
            m = pr < HALFP
            lists[c][w] = (pr[m], nloc_w[m], pr[~m] - HALFP, nloc_w[~m])
            cwlo[c, w] = max(1, math.ceil(int(m.sum()) / P))
            cwhi[c, w] = max(1, math.ceil(int((~m).sum()) / P))
    E = {"lo": np.zeros((NCORES, WPC), np.int64),
         "hi": np.zeros((NCORES, WPC), np.int64)}
    for c in range(NCORES):
        for w in range(WPC):
            E["lo"][c, w] = len(lists[c][w][0])
            E["hi"][c, w] = len(lists[c][w][2])

    # packed per-batch streams: windows concatenated with no alignment;
    # each window processes chunk range [min-core start, max-core end)
    batches = make_batches()
    meta = {}
    for half in ("lo", "hi"):
        mb_list = []
        c0 = nob = 0
        for (w0, nw) in batches:
            cums = np.zeros((NCORES, nw + 1), np.int64)
            cums[:, 1:] = np.cumsum(E[half][:, w0:w0 + nw], axis=1)
            tmax = int(cums[:, nw].max())
            nidx = max(16, -(-tmax // 16) * 16)
            ncols = -(-nidx // 128)
            wins = []
            for i in range(nw):
                s_i = min(int(cums[:, i].min()) // 128, ncols - 1)
                e_i = max(min(-(-int(cums[:, i + 1].max()) // 128), ncols),
                          s_i + 1)
                wins.append((s_i, e_i, nob))
                nob += e_i - s_i
            mb_list.append(dict(c0=c0, ncols=ncols, nidx=nidx, wins=wins,
                                klast=nidx - 128 * (ncols - 1)))
            c0 += ncols
        meta[half] = dict(batches=mb_list, TC=c0, NOC=nob)

    def wrap16(flat):
        # index i -> [i % 16, i // 16], replicated across 128 partitions
        n = len(flat)
        arr = np.zeros((P, n // 16), np.int16)
        arr[:16] = flat.reshape(n // 16, 16).T
        for r in range(1, 8):
            arr[16 * r:16 * (r + 1)] = arr[:16]
        return arr

    per_core = []
    for c in range(NCORES):
        b = c * NB
        streams = {}
        for half, ilo in (("lo", 0), ("hi", 2)):
            m = meta[half]
            idxf = np.zeros(m["TC"] * P, np.int16)
            nof = np.full(m["NOC"] * P, -1.0, np.float32)
            for (w0, nw), mb in zip(batches, m["batches"]):
                base = mb["c0"] * P
                pos = 0
                for i in range(nw):
                    s, nl = lists[c][w0 + i][ilo], lists[c][w0 + i][ilo + 1]
                    ln = len(s)
                    idxf[base + pos:base + pos + ln] = s.astype(np.int16)
                    s_i, e_i, nob_i = mb["wins"][i]
                    pa = np.arange(pos, pos + ln)
                    tgt = (nob_i + pa // P - s_i) * P + pa % P
                    nof[tgt] = nl
                    pos += ln
            streams[f"idx_{half}"] = wrap16(idxf)
            streams[f"no_{half}"] = nof.reshape(m["NOC"], P).T.copy()

        nid = b + np.arange(NBP)
        ok = np.arange(NBP) < NB
        nidc = np.minimum(nid, N_NODES - 1)
        dinv_c = np.where(ok, dinv[nidc], 0.0).astype(np.float32)
        sqd_c = np.where(ok, sqd[nidc], 0.0).astype(np.float16)
        gmin = int(xb[b])
        xbs_c = np.where(ok, xb[nidc] - gmin, 200.0).astype(np.float32)
        assert int(xb[b + NB - 1]) - gmin + 1 <= P
        pools = np.stack(
            [gmin + np.arange(P, dtype=np.float32) - P * b4 for b4 in range(4)],
            axis=1,
        ).astype(np.float32)

        xs_c = x[b:b + NB] * dinv[b:b + NB, None]
        xT = np.zeros((DIM, NBP), np.float16)
        xT[:, :NB] = xs_c.T.astype(np.float16)

        per_core.append(dict(
            xT=xT,
            dinv2d=dinv_c.reshape(WPC, P).T.copy(),
            dinvsq=(dinv_c * dinv_c).reshape(WPC, P).T.copy(),
            sqdT=sqd_c.reshape(1, NBP),
            xbshift=xbs_c.reshape(WPC, P).T.copy(),
            pools=pools,
            **streams,
        ))

    # full scaled-feature table in permuted layout (same for every core)
    xall = (x * dinv[:, None]).astype(np.float16)
    xtab = np.zeros((TPR + 1, NBP), np.float16)
    n = np.arange(N_NODES)
    cc, rr = n // NB, n % NB
    rows = cc * P + rr % P
    cols = (rr // P) * P
    xtab[rows[:, None], cols[:, None] + np.arange(P)[None, :]] = xall
    for pc in per_core:
        pc["xtab"] = xtab

    shared = dict(meta=meta)
    return per_core, shared


def const_inputs(W1, b1, W2, b2, Wh, bh):
    iota = np.tile(np.arange(P, dtype=np.float32)[None, :], (P, 1))
    return dict(
        iota=iota, iota16=iota.astype(np.float16),
        ident16=np.eye(P, dtype=np.float16),
        ident=np.eye(P, dtype=np.float32),
        W1=np.asarray(W1, np.float16), W2=np.asarray(W2, np.float16),
        Wh=np.asarray(Wh, np.float32),
        b1=np.asarray(b1, np.float16).reshape(1, DIM),
        b2=np.asarray(b2, np.float16).reshape(1, DIM),
        bh8=np.asarray(bh, np.float32).reshape(1, DIM_OUT) / NCORES,
        ones512=np.ones((1, NUM_GRAPHS), np.float32),
    )


# ---------------------------------------------------------------- kernel
def build_kernel(shared, single_core=False):
    meta = shared["meta"]
    TClo, TChi = meta["lo"]["TC"], meta["hi"]["TC"]
    NOClo, NOChi = meta["lo"]["NOC"], meta["hi"]["NOC"]

    nc = bacc.Bacc("TRN2", target_bir_lowering=False, debug=False,
                   enable_asserts=False,
                   num_devices=1 if single_core else NCORES)

    # inputs
    d_xT = nc.dram_tensor("xT", [DIM, NBP], fp16, kind="ExternalInput")
    d_idx = {h: nc.dram_tensor(f"idx_{h}", [P, tc * 8], i16,
                               kind="ExternalInput")
             for h, tc in (("lo", TClo), ("hi", TChi))}
    d_no = {h: nc.dram_tensor(f"no_{h}", [P, tc], fp32, kind="ExternalInput")
            for h, tc in (("lo", NOClo), ("hi", NOChi))}
    d_dinv = nc.dram_tensor("dinv2d", [P, WPC], fp32, kind="ExternalInput")
    d_dinvsq = nc.dram_tensor("dinvsq", [P, WPC], fp32, kind="ExternalInput")
    d_xtab = nc.dram_tensor("xtab", [TPR + 1, NBP], fp16,
                            kind="ExternalInput")
    d_sqd = nc.dram_tensor("sqdT", [1, NBP], fp16, kind="ExternalInput")
    d_xbs = nc.dram_tensor("xbshift", [P, WPC], fp32, kind="ExternalInput")
    d_pools = nc.dram_tensor("pools", [P, 4], fp32, kind="ExternalInput")
    d_iota = nc.dram_tensor("iota", [P, P], fp32, kind="ExternalInput")
    d_iota16 = nc.dram_tensor("iota16", [P, P], fp16, kind="ExternalInput")
    d_id16 = nc.dram_tensor("ident16", [P, P], fp16, kind="ExternalInput")
    d_W = [nc.dram_tensor("W1", [DIM, DIM], fp16, kind="ExternalInput"),
           nc.dram_tensor("W2", [DIM, DIM], fp16, kind="ExternalInput")]
    d_b = [nc.dram_tensor("b1", [1, DIM], fp16, kind="ExternalInput"),
           nc.dram_tensor("b2", [1, DIM], fp16, kind="ExternalInput")]
    d_Wh = nc.dram_tensor("Wh", [DIM, DIM_OUT], fp32, kind="ExternalInput")
    d_bh = nc.dram_tensor("bh8", [1, DIM_OUT], fp32, kind="ExternalInput")
    d_ones = nc.dram_tensor("ones512", [1, NUM_GRAPHS], fp32,
                            kind="ExternalInput")

    d_out = nc.dram_tensor("out", [NUM_GRAPHS, DIM_OUT], fp32,
                           kind="ExternalOutput")

    # internal DRAM (layer-2 table in permuted [core*P+p, w*DIM+f] layout;
    # the layer-1 table is the host-provided xtab input)
    tbl = nc.dram_tensor("table1", [TPR + 1, NBP], fp16, addr_space="Shared")
    ag_in = nc.dram_tensor("ag_in1", [P, NBP], fp16)
    ar_in = nc.dram_tensor("ar_in", [NUM_GRAPHS, DIM_OUT], fp16)
    ar_out = nc.dram_tensor("ar_out", [NUM_GRAPHS, DIM_OUT], fp16,
                            addr_space="Shared")

    # gather batches: [(w0, nw, col0_lo, cols_lo, col0_hi, cols_hi)]
    batches = make_batches()

    with tile.TileContext(nc) as tc:
        with tc.tile_pool(name="const", bufs=1) as cst, \
             tc.tile_pool(name="big", bufs=1) as bigp, \
             tc.tile_pool(name="glo", bufs=5) as glo_pool, \
             tc.tile_pool(name="ghi", bufs=5) as ghi_pool, \
             tc.tile_pool(name="cpool", bufs=12) as cpool, \
             tc.tile_pool(name="work", bufs=6) as wk, \
             tc.tile_pool(name="ps_feat", bufs=3, space="PSUM") as ps_feat, \
             tc.tile_pool(name="ps_out", bufs=4, space="PSUM") as ps_out, \
             tc.tile_pool(name="ps_aux", bufs=1, space="PSUM") as ps_aux:

            # ---- constants / inputs to SBUF (feature-phase deps first)
            xT_sb = bigp.tile([DIM, NBP], fp16)
            nc.sync.dma_start(xT_sb[:, 0:7 * P], d_xT[:, 0:7 * P])
            nc.sync.dma_start(xT_sb[:, 7 * P:], d_xT[:, 7 * P:])
            W_sb = []
            for l in range(2):
                t = cst.tile([DIM, DIM], fp16, name=f"W{l}_sb")
                nc.sync.dma_start(t[:], d_W[l][:, :])
                W_sb.append(t)
            dinv_sb = cst.tile([P, WPC], fp32)
            nc.sync.dma_start(dinv_sb[:], d_dinv[:, :])
            dinvsq_sb = cst.tile([P, WPC], fp32)
            nc.sync.dma_start(dinvsq_sb[:], d_dinvsq[:, :])
            idx_sb = {}
            no_sb = {}
            for h, tc_, noc_ in (("lo", TClo, NOClo), ("hi", TChi, NOChi)):
                t = bigp.tile([P, tc_ * 8], i16, name=f"idx{h}_sb")
                nc.sync.dma_start(t[:], d_idx[h][:, :])
                idx_sb[h] = t
                t = bigp.tile([P, noc_], fp32, name=f"no{h}_sb")
                nc.sync.dma_start(t[:], d_no[h][:, :])
                no_sb[h] = t
            sqd_sb = cst.tile([1, NBP], fp16)
            nc.sync.dma_start(sqd_sb[:], d_sqd[:, :])
            xbs_sb = cst.tile([P, WPC], fp32)
            nc.sync.dma_start(xbs_sb[:], d_xbs[:, :])
            pools_sb = cst.tile([P, 4], fp32)
            nc.sync.dma_start(pools_sb[:], d_pools[:, :])
            iota_sb = cst.tile([P, P], fp32)
            nc.sync.dma_start(iota_sb[:], d_iota[:, :])
            iota16_sb = cst.tile([P, P], fp16)
            nc.sync.dma_start(iota16_sb[:], d_iota16[:, :])
            id16_sb = cst.tile([P, P], fp16)
            nc.sync.dma_start(id16_sb[:], d_id16[:, :])
            b_sb = []
            for l in range(2):
                t = cst.tile([1, DIM], fp16, name=f"b{l}_sb")
                nc.sync.dma_start(t[:], d_b[l][:, :])
                b_sb.append(t)
            Wh_sb = cst.tile([DIM, DIM_OUT], fp32)
            nc.sync.dma_start(Wh_sb[:], d_Wh[:, :])
            bh_sb = cst.tile([1, DIM_OUT], fp32)
            nc.sync.dma_start(bh_sb[:], d_bh[:, :])
            ones_sb = cst.tile([1, NUM_GRAPHS], fp32)
            nc.sync.dma_start(ones_sb[:], d_ones[:, :])

            x2s_sb = bigp.tile([P, NBP], fp16, name="x2s")
            x2sT_sb = bigp.tile([P, NBP], fp16, name="x2sT")

            # dummy Ln+Exp up front: forces the all-in-one act-func table
            # (natural_log_exp_and_others) to load once, off the critical path
            dum = cst.tile([1, 1], fp32)
            nc.vector.memset(dum[:], 1.0)
            nc.scalar.activation(out=dum[:], in_=dum[:],
                                 func=mybir.ActivationFunctionType.Ln)
            nc.scalar.activation(out=dum[:], in_=dum[:],
                                 func=mybir.ActivationFunctionType.Exp)

            # subrow gather views: one 256B element per edge
            gview = {}
            for h, base in (("lo", 0), ("hi", NCORES * P // 2)):
                gview[h] = [
                    bass.AP(t[base:, :].tensor, t[base:, :].offset,
                            [[DIM, HALFP + 1], [1, DIM]])
                    for t in (d_xtab, tbl)
                ]

            AGB = [6, 13, 20, 27, 34, 41, 45, 48]

            def ag_flush(w):
                # flush x2s windows to ag_in in groups (big descriptors)
                if w in AGB:
                    w0_ = AGB[AGB.index(w) - 1] + 1 if w != 6 else 0
                    gsl = slice(w0_ * P, (w + 1) * P)
                    nc.sync.dma_start(ag_in[:, gsl], x2s_sb[:, gsl])
                if w == WPC - 1:
                    if single_core:
                        nc.sync.dma_start(tbl[0:P, :], ag_in[:, :])
                    else:
                        nc.gpsimd.collective_compute(
                            "AllGather", mybir.AluOpType.bypass,
                            ins=[ag_in[:, :]],
                            outs=[tbl[0:TPR, :]],
                            replica_groups=[list(range(NCORES))])

            # pooling scatter one-hots depend only on constants: prebuild
            S4 = cst.tile([P, 4, P], fp32)
            for b4 in range(4):
                nc.vector.tensor_scalar(
                    out=S4[:, b4, :], in0=iota_sb[:],
                    scalar1=pools_sb[:, b4:b4 + 1],
                    scalar2=None, op0=mybir.AluOpType.is_equal)

            # ---- aggregation layers
            pool_ps = ps_aux.tile([P, DIM], fp32, space="PSUM", tag="aux")

            for l in range(2):
                for bi, (w0, nw) in enumerate(batches):
                    gt = {}
                    for h, pool_ in (("lo", glo_pool), ("hi", ghi_pool)):
                        mb = meta[h]["batches"][bi]
                        g = pool_.tile([P, mb["ncols"], DIM], fp16,
                                       tag="g", name=f"g{h}_{l}_{w0}")
                        c0 = mb["c0"]
                        nidx = mb["nidx"]
                        nc.gpsimd.dma_gather(
                            out_ap=g[:, :, :], in_ap=gview[h][l],
                            idxs_ap=idx_sb[h][:, 8 * c0:8 * c0 + nidx // 16],
                            num_idxs=nidx, num_idxs_reg=nidx,
                            elem_size=DIM, elem_step=DIM,
                            single_packet=False)
                        gt[h] = g

                    for w in range(w0, w0 + nw):
                        sl = slice(w * P, (w + 1) * P)
                        # poT accumulates [feat x dst] (transposed)
                        po = ps_out.tile([P, DIM], fp32, space="PSUM", tag="po")
                        selfT = xT_sb if l == 0 else x2sT_sb
                        nc.tensor.matmul(out=po[:], lhsT=id16_sb[:],
                                         rhs=selfT[:, sl],
                                         start=True, stop=False)
                        for h in ("lo", "hi"):
                            mb = meta[h]["batches"][bi]
                            s_i, e_i, nob_i = mb["wins"][w - w0]
                            last = h == "hi"
                            for j in range(s_i, e_i):
                                k = (mb["klast"] if j == mb["ncols"] - 1
                                     else P)
                                nocol = nob_i + j - s_i
                                C = cpool.tile([P, P], fp16, tag="C")
                                nc.vector.tensor_scalar(
                                    out=C[0:k, :], in0=iota16_sb[0:k, :],
                                    scalar1=no_sb[h][0:k, nocol:nocol + 1],
                                    scalar2=None,
                                    op0=mybir.AluOpType.is_equal)
                                nc.tensor.matmul(
                                    out=po[:], lhsT=gt[h][0:k, j, 0:DIM],
                                    rhs=C[0:k, :],
                                    start=False,
                                    stop=last and (j == e_i - 1))
                        # epilogue: apply W post-aggregation
                        posb = wk.tile([P, DIM], fp16, tag="posb")
                        nc.scalar.activation(
                            out=posb[:], in_=po[:],
                            func=mybir.ActivationFunctionType.Copy)
                        ph2 = ps_feat.tile([P, DIM], fp32, space="PSUM",
                                           tag="ph")
                        nc.tensor.matmul(out=ph2[:], lhsT=posb[:],
                                         rhs=W_sb[l][:],
                                         start=True, stop=False)
                        nc.tensor.matmul(out=ph2[:],
                                         lhsT=sqd_sb[0:1, sl],
                                         rhs=b_sb[l][:], start=False,
                                         stop=True)
                        if l == 0:
                            # x2s = relu(ph*dinv + b)*dinv = relu(ph*dinv^2
                            #       + sqd*b*dinv^2)   (dinv > 0)
                            nc.scalar.activation(
                                out=x2s_sb[:, sl], in_=ph2[:],
                                func=mybir.ActivationFunctionType.Relu,
                                scale=dinvsq_sb[:, w:w + 1])
                            ptr = ps_feat.tile([P, DIM], fp16, space="PSUM",
                                               tag="ph")
                            nc.tensor.transpose(out=ptr[:],
                                                in_=x2s_sb[:, sl],
                                                identity=id16_sb[:])
                            nc.scalar.activation(
                                out=x2sT_sb[:, sl], in_=ptr[:],
                                func=mybir.ActivationFunctionType.Copy)
                            ag_flush(w)
                        else:
                            xn = wk.tile([P, DIM], fp16, tag="xn")
                            nc.scalar.activation(
                                out=xn[:], in_=ph2[:],
                                func=mybir.ActivationFunctionType.Relu,
                                scale=dinv_sb[:, w:w + 1])
                            Cg = wk.tile([P, P], fp16, tag="Cg")
                            nc.vector.tensor_scalar(
                                out=Cg[:], in0=iota16_sb[:],
                                scalar1=xbs_sb[:, w:w + 1],
                                scalar2=None, op0=mybir.AluOpType.is_equal)
                            nc.tensor.matmul(out=pool_ps[:], lhsT=Cg[:],
                                             rhs=xn[:], start=(w == 0),
                                             stop=(w == WPC - 1))

            # ---- pooling scatter + AllReduce
            pool_sb = wk.tile([P, DIM], fp32)
            nc.vector.tensor_copy(pool_sb[:], pool_ps[:])
            lgi4 = wk.tile([P, 4, DIM_OUT], fp16)
            for b4 in range(4):
                # [feat x graph-block] scatter, then partial logits pre-AR
                pblk = ps_feat.tile([P, P], fp32, space="PSUM", tag="ph")
                nc.tensor.matmul(out=pblk[:], lhsT=pool_sb[:],
                                 rhs=S4[:, b4, :], start=True, stop=True)
                sblk = wk.tile([P, P], fp32, tag="sblk")
                nc.vector.tensor_copy(sblk[:], pblk[:])
                lp = ps_feat.tile([P, DIM_OUT], fp32, space="PSUM", tag="ph")
                nc.tensor.matmul(out=lp[:], lhsT=sblk[:],
                                 rhs=Wh_sb[:], start=True, stop=False)
                nc.tensor.matmul(out=lp[:], lhsT=ones_sb[0:1, 0:P],
                                 rhs=bh_sb[:], start=False, stop=True)
                nc.vector.tensor_copy(lgi4[:, b4, :], lp[:])
            nc.sync.dma_start(
                ar_in[:, :].rearrange("(b p) c -> p b c", p=P), lgi4[:])
            if single_core:
                nc.sync.dma_start(ar_out[:, :], ar_in[:, :])
            else:
                nc.gpsimd.collective_compute(
                    "AllReduce", mybir.AluOpType.add,
                    ins=[ar_in[:, :]], outs=[ar_out[:, :]],
                    replica_groups=[list(range(NCORES))])

            # ---- head: log_softmax on reduced logits
            lg4 = wk.tile([P, 4, DIM_OUT], fp16)
            nc.sync.dma_start(
                lg4[:], ar_out[:, :].rearrange("(b p) c -> p b c", p=P))
            e4 = wk.tile([P, 4, DIM_OUT], fp32)
            se4 = wk.tile([P, 4], fp32)
            lse4 = wk.tile([P, 4], fp32)
            o4 = wk.tile([P, 4, DIM_OUT], fp32)
            # logits are O(5) here, so exp without max-shift is safe
            nc.scalar.activation(out=e4[:].rearrange("p a b -> p (a b)"),
                                 in_=lg4[:].rearrange("p a b -> p (a b)"),
                                 func=mybir.ActivationFunctionType.Exp)
            nc.vector.tensor_reduce(out=se4[:], in_=e4[:],
                                    op=mybir.AluOpType.add,
                                    axis=mybir.AxisListType.X)
            nc.scalar.activation(out=lse4[:], in_=se4[:],
                                 func=mybir.ActivationFunctionType.Ln)
            for b4 in range(4):
                nc.vector.tensor_scalar(out=o4[:, b4, :], in0=lg4[:, b4, :],
                                        scalar1=lse4[:, b4:b4 + 1],
                                        scalar2=None,
                                        op0=mybir.AluOpType.subtract)
            nc.sync.dma_start(d_out[:, :].rearrange("(b p) f -> p b f", p=P),
                              o4[:])

    nc.compile()
    return nc


# ---------------------------------------------------------------- entry
def kernel(x, edge_index, x_batch, W1, b1, W2, b2, Wh, bh):
    """Full-input GCN kernel: shards nodes/edges across 8 NeuronCores."""
    from concourse.bass_utils import run_bass_kernel_spmd

    per_core, shared = preprocess(x, edge_index, x_batch)
    consts = const_inputs(W1, b1, W2, b2, Wh, bh)
    in_maps = [{**pc, **consts} for pc in per_core]
    nc = build_kernel(shared)
    declared = set()
    for alloc in nc.m.functions[0].allocations:
        if isinstance(alloc, mybir.MemoryLocationSet) and \
                alloc.kind == "ExternalInput":
            declared.add(alloc.memorylocations[0].name)
    in_maps = [{k: v for k, v in m.items() if k in declared} for m in in_maps]
    res = run_bass_kernel_spmd(nc, in_maps, core_ids=list(range(NCORES)))
    return np.asarray(res.results[0]["out"], dtype=np.float32)
